# revision 14
# baseline (speedup 1.0000x reference)
"""CapsuleLayer dynamic-routing kernel for Trainium2 (Bass/Tile), SPMD over 8 cores.

Problem (per full input):
  x:  [256, 1152, 8]   route_weights: [10, 1152, 8, 16]
  priors[c,b,n,o] = sum_i x[b,n,i] * W[c,n,i,o]
  3 routing iterations; logits along o are constant =>
  probs are per-(c,b,n) scalars.  out: [10, 256, 1, 1, 16]

Math (per core, b = 32 local batch):
  logits_t[c,b,n] = priors[c,b,n,:] . a_t[c,b,:], a_t = sum of previous
  squashed outputs.  l = sum_i x * V,  V = sum_o W a  (PE matmul, k=(c,o)).
  s_raw[c,b,o] = sum_{n,i} e^l x W  (PE matmul over k=(i,n)).
  squash via shift-invariant form  v = s_raw * sqrt(snr) / (Z^2 + snr),
  snr = |s_raw|^2, Z = sum_n e^l  (probs normalizer folded in).

v4 changes over v3 (which measured 161.8us):
  - head: wk DMA in 4 pieces consumed progressively by the iter-0 s-chain;
    wt per-group pieces land just-in-time so iter-1 V-matmuls start ~13us
    instead of ~34us.
  - squash smalls rebuilt around exp/ln:  f = exp(0.5 ln snr - ln(Z^2+snr))
    -> no DVE reciprocal (was 11us of Vector), no sqrt table set, Z^2+snr
    accumulated into PSUM by a K=1 matmul, ln/exp straight from PSUM.
  - f pre-masked before the selc broadcast matmul, so the a-accumulator
    needs no per-iteration masking (saves 2 DVE ops per squash).
  - xe written in-place into the xv tile (saves 4.7MB SBUF + WAR clarity).
  - capsules in two groups of 5, packed (c,o) x (c',b); off-diag blocks of
    the a accumulator are zero (masked f), making the packed V-matmul exact.
"""

import os
from contextlib import ExitStack

import numpy as np

B, N, CI, CO, NCAPS = 256, 1152, 8, 16, 10
NCORES = 8
BL = B // NCORES          # 32 batch per core
NB = N // 128             # 9 n-blocks
NCH = CI * NB             # 72 k-chunks, j = i*9 + nb
CG = 5                    # capsules per group
KO = CG * CO              # 80 partition rows in (c,o) layout
FB = CG * BL              # 160 free cols in (c',b) layout
NUM_ITERATIONS = 3

# fp32 const-blob column map
EXPB1, EXPB2 = 0, 1       # exp shifts (0.0 / -9.5)
BD0 = 2                   # bdones [80, 5]  cols 2:7
SEL0 = 7                  # selc  [5, 80]   cols 7:87
ONE1C = 87                # ones_1c [1, 5]  cols 87:92
MASK50 = 92               # mask5 [5, 2*160] cols 92:412
NSQ = 412                 # ln-bias N^2 for iter-0 squash
C32_COLS = 416

# bf16 const-blob: col 0 = ones_nl, cols 1.. = x duplicate (iter-0 rhs)
CB16_COLS = 1 + NCH * BL

_compiled = None


def _build():
    import concourse.bacc as bacc
    import concourse.mybir as mybir
    import concourse.tile as tile

    fp32 = mybir.dt.float32
    fp16 = mybir.dt.float16
    bf16 = mybir.dt.bfloat16
    Alu = mybir.AluOpType
    Act = mybir.ActivationFunctionType

    nc = bacc.Bacc("TRN2", target_bir_lowering=False, debug=False)
    wkb_d = nc.dram_tensor("wkb", [128, NCH * NCAPS * CO], bf16,
                           kind="ExternalInput")
    cb16_d = nc.dram_tensor("cb16", [128, CB16_COLS], bf16,
                            kind="ExternalInput")
    xf_d = nc.dram_tensor("xf", [128, NCH * BL], fp16, kind="ExternalInput")
    wt_d = nc.dram_tensor("wt", [KO, 2 * NCH * 128], fp16, kind="ExternalInput")
    c32_d = nc.dram_tensor("c32", [128, C32_COLS], fp32, kind="ExternalInput")
    o_d = nc.dram_tensor("out", [KO, 2, FB], fp32, kind="ExternalOutput")
    DBG = bool(int(os.environ.get("CAPS_DBG", "0")))
    if DBG:
        dbg_elt_d = nc.dram_tensor("dbg_elt", [128, 2 * 2 * NB * FB], bf16,
                                   kind="ExternalOutput")
        dbg_ssb_d = nc.dram_tensor("dbg_ssb", [KO, 3 * 2 * FB], fp32,
                                   kind="ExternalOutput")
        dbg_zz_d = nc.dram_tensor("dbg_zz", [1, 2 * 2 * FB], fp32,
                                  kind="ExternalOutput")
        dbg_f_d = nc.dram_tensor("dbg_f", [CG, 3 * 2 * FB], fp32,
                                 kind="ExternalOutput")

    with tile.TileContext(nc) as tc, ExitStack() as ctx:
        P = ctx.enter_context(tc.tile_pool(name="persist", bufs=1))
        vbp = ctx.enter_context(tc.tile_pool(name="vbp", bufs=2))
        xvp = ctx.enter_context(tc.tile_pool(name="xvp", bufs=1))
        xpp = ctx.enter_context(tc.tile_pool(name="xpp", bufs=2))
        smallp = ctx.enter_context(tc.tile_pool(name="small", bufs=1))
        psv = ctx.enter_context(tc.tile_pool(name="psv", bufs=2, space="PSUM"))
        pss = ctx.enter_context(tc.tile_pool(name="pss", bufs=1, space="PSUM"))
        psq = ctx.enter_context(tc.tile_pool(name="psq", bufs=1, space="PSUM"))

        # ---- persistent SBUF ----
        wkt = P.tile([128, NCH, NCAPS * CO], bf16)
        cb16 = P.tile([128, CB16_COLS], bf16)
        xf = P.tile([128, NCH, BL], fp16)
        c32 = P.tile([128, C32_COLS], fp32)
        wt = P.tile([KO, 2, NCH, 128], fp16)
        xc = P.tile([128, NCH, CG, BL], fp16)  # x replicated over c'
        a_bf = P.tile([KO, 2, FB], fp16)
        elt = P.tile([128, 2, NB, FB], bf16)       # e^l per group

        wk = wkt[:, :, :]                                  # [128, 72, 160]
        xh = cb16[:, 1:].rearrange("p (j b) -> p j b", j=NCH)
        ones16 = cb16[:, 0:1]                              # [128, 1] bf16
        expb1 = c32[:, EXPB1:EXPB1 + 1]
        expb2 = c32[:, EXPB2:EXPB2 + 1]
        bdones = c32[0:KO, BD0:BD0 + CG]                   # [80, 5] fp32
        selc = c32[0:CG, SEL0:SEL0 + KO]                   # [5, 80] fp32
        ones1c = c32[0:1, ONE1C:ONE1C + CG]                # [1, 5] fp32
        mask5 = c32[0:CG, MASK50:MASK50 + 2 * FB]          # [5, 320] fp32
        nsqb = c32[0:CG, NSQ:NSQ + 1]                      # [5, 1] = N^2

        def xbc(j):
            """bf16 x chunk [128, 5, 32] broadcast over capsules (iter-0)."""
            return xh[:, j, :].unsqueeze(1).broadcast_to([128, CG, BL])

        # ---- input DMAs (two HWDGE queues) ----
        # qB (scalar): consts, x dup, x, then wt per-group pieces.
        # qA (sync):   wk in 4 pieces, consumed progressively by iter-0.
        nc.scalar.dma_start(c32[:], c32_d[:])
        nc.scalar.dma_start(cb16[:], cb16_d[:])
        WKP = 4
        JP = NCH // WKP
        for p in range(WKP):
            nc.sync.dma_start(
                wkt[:, p * JP:(p + 1) * JP, :],
                wkb_d[:, p * JP * NCAPS * CO:(p + 1) * JP * NCAPS * CO]
                .rearrange("p (j c) -> p j c", j=JP))
        nc.scalar.dma_start(xf.rearrange("p j b -> p (j b)"), xf_d[:])
        # wt group 0 in 2 pieces (consumed by iter-1 V-matmuls), then g1
        H = NCH // 2
        nc.scalar.dma_start(wt[:, 0, 0:H, :].rearrange("p j n -> p (j n)"),
                            wt_d[:, 0:H * 128])
        nc.scalar.dma_start(wt[:, 0, H:, :].rearrange("p j n -> p (j n)"),
                            wt_d[:, H * 128:NCH * 128])
        nc.scalar.dma_start(wt[:, 1, :, :].rearrange("p j n -> p (j n)"),
                            wt_d[:, NCH * 128:])

        for cr in range(CG):
            nc.vector.tensor_scalar_mul(xc[:, :, cr, :], xf[:, :, :], 1.0)

        a_cur = [None, None]   # fp32 cumulative a per group (smallp tiles)

        def squash(it, g, ssb, zz, smt):
            """v = s_raw * sqrt(snr) / (Z^2 + snr); writes a_bf or output.

            ssb: [KO, FB] fp32 SBUF raw s.  zz: [1, FB] fp32 Z^2 (None at it 0).
            smt: [128, 512] fp32 PSUM bank shared with this group's z_pass.
            """
            s2 = smallp.tile([KO, FB], fp32, tag="s2")
            nc.vector.tensor_tensor(s2[:], ssb, ssb, Alu.mult)
            snp = smt[0:CG, 0:FB]
            nc.tensor.matmul(snp, bdones, s2[:], start=True, stop=True)
            u = smallp.tile([CG, FB], fp32, tag="u")
            nc.scalar.activation(u[:], snp, Act.Ln)
            w = smallp.tile([CG, FB], fp32, tag="w")
            if it == 0:
                nc.scalar.activation(w[:], snp, Act.Ln, bias=nsqb)
            else:
                snpz = smt[0:CG, FB:2 * FB]
                nc.tensor.matmul(snpz, bdones, s2[:], start=True, stop=False)
                nc.tensor.matmul(snpz, ones1c, zz, start=False, stop=True)
                nc.scalar.activation(w[:], snpz, Act.Ln)
            h = smallp.tile([CG, FB], fp32, tag="h")
            nc.vector.scalar_tensor_tensor(h[:], u[:], 0.5, w[:],
                                           Alu.mult, Alu.subtract)
            f = smallp.tile([CG, FB], fp32, tag="f")
            nc.scalar.activation(f[:], h[:], Act.Exp)
            if DBG:
                k = it * 2 + g
                nc.sync.dma_start(dbg_ssb_d[:, k * FB:(k + 1) * FB], ssb)
                nc.sync.dma_start(dbg_f_d[:, k * FB:(k + 1) * FB], f[:])
                if zz is not None:
                    kz = (it - 1) * 2 + g
                    nc.sync.dma_start(dbg_zz_d[:, kz * FB:(kz + 1) * FB],
                                      zz[:])
            fm = smallp.tile([CG, FB], fp32, tag="fm")
            nc.vector.tensor_tensor(fm[:], f[:],
                                    mask5[:, g * FB:(g + 1) * FB], Alu.mult)
            frp = smt[0:KO, 2 * FB:3 * FB]
            nc.tensor.matmul(frp, selc, fm[:], start=True, stop=True)
            if it == NUM_ITERATIONS - 1:
                ov = smallp.tile([KO, FB], fp32, tag="ov")
                nc.vector.tensor_tensor(ov[:], ssb, frp, Alu.mult)
                nc.sync.dma_start(o_d[:, g, :], ov[:])
            elif it == 0:
                aq = smallp.tile([KO, FB], fp32, tag="aq0" + str(g))
                nc.vector.tensor_tensor(aq[:], ssb, frp, Alu.mult)
                a_cur[g] = aq
                nc.scalar.copy(a_bf[:, g, :], aq[:])
            else:
                ov = smallp.tile([KO, FB], fp32, tag="ov")
                nc.vector.tensor_tensor(ov[:], ssb, frp, Alu.mult)
                aq = smallp.tile([KO, FB], fp32, tag="aq1" + str(g))
                nc.vector.tensor_tensor(aq[:], a_cur[g][:], ov[:], Alu.add)
                a_cur[g] = aq
                nc.scalar.copy(a_bf[:, g, :], aq[:])

        # ---- PE warm-up: dummy matmuls on c32 while input DMAs stream;
        # keeps HAM at K=8/8 so iter-0 runs at 2.4 GHz ----
        smw = psq.tile([128, 512], fp32, tag="sq", name="sqw")
        for r in range(40):
            nc.tensor.matmul(smw[0:80, 0:96], c32[:, 0:80], c32[:, 0:96],
                             start=True, stop=True)

        # ================= iteration 0 =================
        # s0[(c,o),(c',b)] = sum_j W^T x  (x broadcast over c'), both groups
        # into one PSUM bank; one accumulation chain per bank at a time.
        sps0 = pss.tile([KO, 2 * FB], fp32, tag="spsum", name="sps0")
        for g in (0, 1):
            for j in range(NCH):
                nc.tensor.matmul(sps0[:, g * FB:(g + 1) * FB],
                                 wk[:, j, g * KO:(g + 1) * KO], xbc(j),
                                 start=(j == 0), stop=(j == NCH - 1))
            ssb = smallp.tile([KO, FB], fp32, tag="ssb" + str(g))
            nc.scalar.copy(ssb[:], sps0[:, g * FB:(g + 1) * FB])
            smt = psq.tile([128, 512], fp32, tag="sq", name="sq_i0" + str(g))
            squash(0, g, ssb[:], None, smt)

        # ================= iterations 1, 2 =================
        sps_cur = [None]

        def l_pass(g, it, filler=None):
            xv = xvp.tile([128, CI, NB * FB], fp16, tag="xv",
                          name="xv" + str(it) + str(g))
            for i in range(CI):
                vb = psv.tile([128, 3, 512], fp32, tag="vb")
                for nbg in range(3):
                    for k in range(3):
                        j = i * NB + nbg * 3 + k
                        nc.tensor.matmul(vb[:, nbg, k * FB:(k + 1) * FB],
                                         wt[:, g, j, :], a_bf[:, g, :],
                                         start=True, stop=True)
                vbf = vbp.tile([128, NB * FB], fp16, tag="vbf")
                nc.scalar.copy(
                    vbf.rearrange("p (a b) -> p a b", a=3),
                    vb[:, :, 0:3 * FB])
                # xv_i = x * V_i, contiguous fp16 at 2x
                nc.vector.tensor_tensor(
                    xv[:, i, :].rearrange("p (a f) -> p a f", a=NB),
                    xc[:, i * NB:(i + 1) * NB, :, :]
                    .rearrange("p j c b -> p j (c b)"),
                    vbf.rearrange("p (a f) -> p a f", a=NB),
                    Alu.mult)
                if filler is not None:
                    filler(i)   # independent DVE work fills the evac wait
            # tree-add over i: [8, 9*160]
            xvi = xv.rearrange("p i f -> p (i f)") \
                    .rearrange("p (i f) -> p i f", i=CI)
            t1 = smallp.tile([128, 4, NB * FB], fp16, tag="t1")
            nc.vector.tensor_tensor(t1[:], xvi[:, 0:4, :], xvi[:, 4:8, :],
                                    Alu.add)
            nc.vector.tensor_tensor(xvi[:, 0:2, :], t1[:, 0:2, :],
                                    t1[:, 2:4, :], Alu.add)
            nc.vector.tensor_tensor(xvi[:, 2, :], xvi[:, 0, :], xvi[:, 1, :],
                                    Alu.add)
            nc.scalar.activation(
                elt[:, g, :, :].rearrange("p a b -> p (a b)"), xvi[:, 2, :],
                Act.Exp, bias=(expb1 if it == 1 else expb2))
            if DBG:
                k = (it - 1) * 2 + g
                nc.sync.dma_start(
                    dbg_elt_d[:, k * NB * FB:(k + 1) * NB * FB],
                    elt[:, g, :, :].rearrange("p a b -> p (a b)"))
            return xv

        def z_pass(g, smt):
            """Z = sum_n e^l via PE ones-matmul; returns Z^2 in SBUF fp32.

            Uses the frp col-region of smt; zz is extracted before squash's
            frp matmul touches it (WAR inside the same tile).
            """
            zp = smt[0:1, 2 * FB:3 * FB]
            for nb in range(NB):
                nc.tensor.matmul(zp, ones16, elt[:, g, nb, :],
                                 start=(nb == 0), stop=(nb == NB - 1))
            zz = smallp.tile([1, FB], fp32, tag="zz" + str(g), name="zz")
            nc.scalar.activation(zz[:], zp, Act.Square)
            return zz

        def xe_op(g, xp, i):
            # xe_i = x * e^l, contiguous (fp16 x bf16 runs at 2x); bf16 out
            # because x*e^l can exceed the fp16 range
            nc.vector.tensor_tensor(
                xp[:, i * NB:(i + 1) * NB, :],
                xc[:, i * NB:(i + 1) * NB, :, :]
                .rearrange("p j c b -> p j (c b)"),
                elt[:, g, :, :], Alu.mult)

        def s_mm(g, xp):
            for j in range(NCH):
                nc.tensor.matmul(
                    sps_cur[0][:, g * FB:(g + 1) * FB],
                    wk[:, j, g * KO:(g + 1) * KO], xp[:, j, :],
                    start=(j == 0), stop=(j == NCH - 1))
            ssb = smallp.tile([KO, FB], fp32, tag="ssb" + str(g))
            nc.scalar.copy(ssb[:], sps_cur[0][:, g * FB:(g + 1) * FB])
            return ssb

        for it in (1, 2):
            sps_cur[0] = pss.tile([KO, 2 * FB], fp32, tag="spsum",
                                  name="sps")
            # V0,V1 dense on PE; xe0 fills l1's evac-paced DVE gaps;
            # per-group squash lets a_bf0 be ready before the next
            # iteration's V0 matmuls
            l_pass(0, it)
            xp0 = xpp.tile([128, NCH, FB], bf16, tag="xp", name="xp0")
            l_pass(1, it, filler=lambda i: xe_op(0, xp0, i))
            smt0 = psq.tile([128, 512], fp32, tag="sq", name="sq0")
            zz0 = z_pass(0, smt0)
            ssb0 = s_mm(0, xp0)
            squash(it, 0, ssb0[:], zz0[:], smt0)
            xp1 = xpp.tile([128, NCH, FB], bf16, tag="xp", name="xp1")
            for i in range(CI):
                xe_op(1, xp1, i)
            smt1 = psq.tile([128, 512], fp32, tag="sq", name="sq1")
            zz1 = z_pass(1, smt1)
            ssb1 = s_mm(1, xp1)
            squash(it, 1, ssb1[:], zz1[:], smt1)

    nc.compile()
    return nc


def _get_compiled():
    global _compiled
    if _compiled is None:
        _compiled = _build()
    return _compiled


def _make_consts():
    import ml_dtypes
    c32 = np.zeros((128, C32_COLS), dtype=np.float32)
    for q in range(CG):
        c32[q * CO:(q + 1) * CO, BD0 + q] = 1.0
        c32[q, SEL0 + q * CO:SEL0 + (q + 1) * CO] = 1.0
        for g in range(2):
            c32[q, MASK50 + g * FB + q * BL:MASK50 + g * FB + (q + 1) * BL] = 1.0
    c32[0, ONE1C:ONE1C + CG] = 1.0
    c32[:, EXPB1] = 0.0
    c32[:, EXPB2] = -9.5   # keeps snr within the Ln table's accurate range
    c32[:, NSQ] = float(N) * float(N)
    return c32


def _prep_w(route_weights: np.ndarray):
    w = np.ascontiguousarray(route_weights, dtype=np.float32)
    w5 = w.reshape(NCAPS, NB, 128, CI, CO)
    import ml_dtypes
    wk = np.ascontiguousarray(
        w5.transpose(2, 3, 1, 0, 4).reshape(128, NCH * NCAPS * CO)
        .astype(ml_dtypes.bfloat16))
    wt = np.ascontiguousarray(
        w5.reshape(2, CG, NB, 128, CI, CO)
        .transpose(1, 5, 0, 4, 2, 3).reshape(KO, 2 * NCH * 128)
        .astype(np.float16))
    return wk, wt


def _prep_x_shard(xs: np.ndarray):
    xf = np.ascontiguousarray(
        xs.reshape(BL, NB, 128, CI).transpose(2, 3, 1, 0).reshape(128, NCH * BL))
    return xf.astype(np.float16)


def _extract_out(raw: np.ndarray) -> np.ndarray:
    """raw [KO, 2, FB] -> [NCAPS, BL, CO] diagonal blocks."""
    out = np.empty((NCAPS, BL, CO), dtype=np.float32)
    for c in range(NCAPS):
        g, cl = divmod(c, CG)
        out[c] = raw[cl * CO:(cl + 1) * CO, g, cl * BL:(cl + 1) * BL].T
    return out


def kernel(x: np.ndarray, route_weights: np.ndarray) -> np.ndarray:
    from concourse.bass_utils import run_bass_kernel_spmd
    import ml_dtypes

    nc = _get_compiled()
    x = np.ascontiguousarray(x, dtype=np.float32)
    wk, wt = _prep_w(route_weights)
    c32 = _make_consts()
    in_maps = []
    for ci in range(NCORES):
        xh16 = _prep_x_shard(x[ci * BL:(ci + 1) * BL])
        cb16 = np.zeros((128, CB16_COLS), dtype=ml_dtypes.bfloat16)
        cb16[:, 0] = 1.0
        cb16[:, 1:] = xh16.astype(ml_dtypes.bfloat16)
        in_maps.append({"wkb": wk, "cb16": np.ascontiguousarray(cb16),
                        "xf": xh16, "wt": wt, "c32": c32})
    tdir = os.environ.get("CAPS_TRACE_DIR") or None
    if tdir:
        os.makedirs(tdir, exist_ok=True)
    res = run_bass_kernel_spmd(
        nc, in_maps, list(range(NCORES)), tmpdir=tdir,
        trace=bool(int(os.environ.get("CAPS_TRACE", "0"))))
    kernel.last_res = res
    outs = [_extract_out(res.results[ci]["out"]) for ci in range(NCORES)]
    full = np.concatenate(outs, axis=1)          # [10, 256, 16]
    if res.exec_time_ns is not None:
        kernel.last_exec_time_ns = res.exec_time_ns
    return full[:, :, None, None, :].astype(np.float32)


kernel.last_exec_time_ns = None


# revision 17
# speedup vs baseline: 1.0168x; 1.0168x over previous
"""CapsuleLayer dynamic-routing kernel for Trainium2 (Bass/Tile), SPMD over 8 cores.

Problem (per full input):
  x:  [256, 1152, 8]   route_weights: [10, 1152, 8, 16]
  priors[c,b,n,o] = sum_i x[b,n,i] * W[c,n,i,o]
  3 routing iterations; logits along o are constant =>
  probs are per-(c,b,n) scalars.  out: [10, 256, 1, 1, 16]

Math (per core, b = 32 local batch):
  logits_t[c,b,n] = priors[c,b,n,:] . a_t[c,b,:], a_t = sum of previous
  squashed outputs.  l = sum_i x * V,  V = sum_o W a  (PE matmul, k=(c,o)).
  s_raw[c,b,o] = sum_{n,i} e^l x W  (PE matmul over k=(i,n)).
  squash via shift-invariant form  v = s_raw * sqrt(snr) / (Z^2 + snr),
  snr = |s_raw|^2, Z = sum_n e^l  (probs normalizer folded in).

v4.1 over v3 (161.8us):
  - all 16-bit tensors fp16 (W gains 3 mantissa bits over bf16); the it-2
    exp shift is -15 so x*e^l stays in fp16 range; exp then runs 2x.
  - squash rebuilt on exp/ln: f = exp(0.5 ln snr - ln(Z^2+snr)); no DVE
    reciprocal (was 11us), no sqrt table.  Ln and Exp live in different
    ACT table sets, so squashes are split into a PRE phase (both groups'
    Lns back-to-back) and FIN phase (both Exps) -> 2 table swaps per
    iteration instead of 4+.
  - s2 = s_raw^2 via scalar Square directly from PSUM (off the DVE).
  - Z^2+snr accumulated into PSUM by a K=1 matmul; iter-0 uses a constant
    N^2 ln-bias instead.
  - f pre-masked before the selc broadcast matmul -> no a-masking ops.
  - head: wk DMA in 4 pieces consumed progressively by the iter-0 s-chain,
    wt per-group pieces land just-in-time for iter 1.
  - keep-warm dummy matmuls over squash FIN phases so HAM stays at 8/8.
"""

import os
from contextlib import ExitStack

import numpy as np

B, N, CI, CO, NCAPS = 256, 1152, 8, 16, 10
NCORES = 8
BL = B // NCORES          # 32 batch per core
NB = N // 128             # 9 n-blocks
NCH = CI * NB             # 72 k-chunks, j = i*9 + nb
CG = 5                    # capsules per group
KO = CG * CO              # 80 partition rows in (c,o) layout
FB = CG * BL              # 160 free cols in (c',b) layout
NUM_ITERATIONS = 3

# fp32 const-blob column map
EXPB1, EXPB2 = 0, 1       # exp shifts (0.0 / -15.0)
BD0 = 2                   # bdones [80, 5]  cols 2:7
SEL0 = 7                  # selc  [5, 80]   cols 7:87
ONE1C = 87                # ones_1c [1, 5]  cols 87:92
MASK50 = 92               # mask5 [5, 2*160] cols 92:412
NSQ = 412                 # ln-bias N^2 for iter-0 squash
C32_COLS = 416

CB16_COLS = 8             # fp16 consts: col 0 = ones_nl

_compiled = None


def _build():
    import concourse.bacc as bacc
    import concourse.mybir as mybir
    import concourse.tile as tile

    fp32 = mybir.dt.float32
    fp16 = mybir.dt.float16
    Alu = mybir.AluOpType
    Act = mybir.ActivationFunctionType

    nc = bacc.Bacc("TRN2", target_bir_lowering=False, debug=False)
    wkb_d = nc.dram_tensor("wkb", [128, NCH * NCAPS * CO], fp16,
                           kind="ExternalInput")
    cb16_d = nc.dram_tensor("cb16", [128, CB16_COLS], fp16,
                            kind="ExternalInput")
    xf_d = nc.dram_tensor("xf", [128, NCH * BL], fp16, kind="ExternalInput")
    wt_d = nc.dram_tensor("wt", [KO, 2 * NCH * 128], fp16, kind="ExternalInput")
    c32_d = nc.dram_tensor("c32", [128, C32_COLS], fp32, kind="ExternalInput")
    o_d = nc.dram_tensor("out", [KO, 2, FB], fp32, kind="ExternalOutput")

    with tile.TileContext(nc) as tc, ExitStack() as ctx:
        P = ctx.enter_context(tc.tile_pool(name="persist", bufs=1))
        vbp = ctx.enter_context(tc.tile_pool(name="vbp", bufs=2))
        xvp = ctx.enter_context(tc.tile_pool(name="xvp", bufs=1))
        xpp = ctx.enter_context(tc.tile_pool(name="xpp", bufs=2))
        smallp = ctx.enter_context(tc.tile_pool(name="small", bufs=1))
        psv = ctx.enter_context(tc.tile_pool(name="psv", bufs=2, space="PSUM"))
        pss = ctx.enter_context(tc.tile_pool(name="pss", bufs=1, space="PSUM"))
        psq = ctx.enter_context(tc.tile_pool(name="psq", bufs=1, space="PSUM"))

        # ---- persistent SBUF ----
        wkt = P.tile([128, NCH, NCAPS * CO], fp16)
        cb16 = P.tile([128, CB16_COLS], fp16)
        xf = P.tile([128, NCH, BL], fp16)
        c32 = P.tile([128, C32_COLS], fp32)
        wt = P.tile([KO, 2, NCH, 128], fp16)
        xc = P.tile([128, NCH, CG, BL], fp16)  # x replicated over c'
        a_bf = P.tile([KO, 2, FB], fp16)
        elt = P.tile([128, 2, NB, FB], fp16)       # e^l per group

        wk = wkt[:, :, :]                                  # [128, 72, 160]
        ones16 = cb16[:, 0:1]                              # [128, 1] fp16
        expb1 = c32[:, EXPB1:EXPB1 + 1]
        expb2 = c32[:, EXPB2:EXPB2 + 1]
        bdones = c32[0:KO, BD0:BD0 + CG]                   # [80, 5] fp32
        selc = c32[0:CG, SEL0:SEL0 + KO]                   # [5, 80] fp32
        ones1c = c32[0:1, ONE1C:ONE1C + CG]                # [1, 5] fp32
        mask5 = c32[0:CG, MASK50:MASK50 + 2 * FB]          # [5, 320] fp32
        nsqb = c32[0:CG, NSQ:NSQ + 1]                      # [5, 1] = N^2

        def xbc(j):
            """fp16 x chunk [128, 5, 32] broadcast over capsules (iter-0)."""
            return xf[:, j, :].unsqueeze(1).broadcast_to([128, CG, BL])

        # ---- input DMAs (two HWDGE queues) ----
        nc.scalar.dma_start(c32[:], c32_d[:])
        nc.scalar.dma_start(cb16[:], cb16_d[:])
        WKP = 4
        JP = NCH // WKP
        for p in range(WKP):
            nc.sync.dma_start(
                wkt[:, p * JP:(p + 1) * JP, :],
                wkb_d[:, p * JP * NCAPS * CO:(p + 1) * JP * NCAPS * CO]
                .rearrange("p (j c) -> p j c", j=JP))
        nc.scalar.dma_start(xf.rearrange("p j b -> p (j b)"), xf_d[:])
        H = NCH // 2
        nc.scalar.dma_start(wt[:, 0, 0:H, :].rearrange("p j n -> p (j n)"),
                            wt_d[:, 0:H * 128])
        nc.scalar.dma_start(wt[:, 0, H:, :].rearrange("p j n -> p (j n)"),
                            wt_d[:, H * 128:NCH * 128])
        nc.scalar.dma_start(wt[:, 1, :, :].rearrange("p j n -> p (j n)"),
                            wt_d[:, NCH * 128:])

        for cr in range(CG):
            nc.vector.tensor_scalar_mul(xc[:, :, cr, :], xf[:, :, :], 1.0)

        a_cur = [None, None]   # fp32 cumulative a per group
        sq_st = [None, None]   # per-group squash state between PRE and FIN

        def squash_pre(it, g, sps_ap, smt):
            """s2, snr(+Z^2) matmuls, and both Lns for group g.

            sps_ap: [KO, FB] fp32 PSUM raw s.  smt: shared small PSUM bank
            (zp already consumed).  Leaves (u, w, ssb) for squash_fin.
            """
            s2 = smallp.tile([KO, FB], fp32, tag="s2" + str(g))
            nc.scalar.activation(s2[:], sps_ap, Act.Square)
            ssb = smallp.tile([KO, FB], fp32, tag="ssb" + str(g))
            nc.scalar.copy(ssb[:], sps_ap)
            snp = smt[0:CG, 0:FB]
            nc.tensor.matmul(snp, bdones, s2[:], start=True, stop=True)
            u = smallp.tile([CG, FB], fp32, tag="u" + str(g))
            w = smallp.tile([CG, FB], fp32, tag="w" + str(g))
            if it == 0:
                nc.scalar.activation(u[:], snp, Act.Ln)
                nc.scalar.activation(w[:], snp, Act.Ln, bias=nsqb)
            else:
                zz = sq_st[g][0]
                snpz = smt[0:CG, FB:2 * FB]
                nc.tensor.matmul(snpz, bdones, s2[:], start=True, stop=False)
                nc.tensor.matmul(snpz, ones1c, zz[:], start=False, stop=True)
                nc.scalar.activation(u[:], snp, Act.Ln)
                nc.scalar.activation(w[:], snpz, Act.Ln)
            sq_st[g] = (u, w, ssb, smt)

        def squash_fin(it, g):
            """f = exp(0.5u - w), mask, broadcast, v; writes a_bf or out."""
            u, w, ssb, smt = sq_st[g]
            h = smallp.tile([CG, FB], fp32, tag="h" + str(g))
            nc.vector.scalar_tensor_tensor(h[:], u[:], 0.5, w[:],
                                           Alu.mult, Alu.subtract)
            f = smallp.tile([CG, FB], fp32, tag="f" + str(g))
            nc.scalar.activation(f[:], h[:], Act.Exp)
            fm = smallp.tile([CG, FB], fp32, tag="fm" + str(g))
            nc.vector.tensor_tensor(fm[:], f[:],
                                    mask5[:, g * FB:(g + 1) * FB], Alu.mult)
            frp = smt[0:KO, 2 * FB:3 * FB]
            nc.tensor.matmul(frp, selc, fm[:], start=True, stop=True)
            if it == NUM_ITERATIONS - 1:
                ov = smallp.tile([KO, FB], fp32, tag="ov" + str(g))
                nc.vector.tensor_tensor(ov[:], ssb[:], frp, Alu.mult)
                nc.sync.dma_start(o_d[:, g, :], ov[:])
            elif it == 0:
                aq = smallp.tile([KO, FB], fp32, tag="aq0" + str(g))
                nc.vector.tensor_tensor(aq[:], ssb[:], frp, Alu.mult)
                a_cur[g] = aq
                nc.scalar.copy(a_bf[:, g, :], aq[:])
            else:
                ov = smallp.tile([KO, FB], fp32, tag="ov" + str(g))
                nc.vector.tensor_tensor(ov[:], ssb[:], frp, Alu.mult)
                aq = smallp.tile([KO, FB], fp32, tag="aq1" + str(g))
                nc.vector.tensor_tensor(aq[:], a_cur[g][:], ov[:], Alu.add)
                a_cur[g] = aq
                nc.scalar.copy(a_bf[:, g, :], aq[:])

        def warm(smt, n):
            """Dummy matmuls (into unused cols of the small PSUM bank) that
            keep HAM at K=8/8 over PE-idle windows."""
            for r in range(n):
                nc.tensor.matmul(smt[0:80, 3 * FB:3 * FB + 32], c32[:, 0:80],
                                 c32[:, 0:32], start=True, stop=True)

        # ---- PE warm-up while input DMAs stream ----
        smt_w = psq.tile([128, 512], fp32, tag="sq", name="sq_warm")
        warm(smt_w, 60)

        # ================= iteration 0 =================
        # s0[(c,o),(c',b)] = sum_j W^T x  (x broadcast over c'), both groups
        # into one PSUM bank; one accumulation chain per bank at a time.
        sps0 = pss.tile([KO, 2 * FB], fp32, tag="spsum", name="sps0")
        smt_i0 = [None, None]
        for g in (0, 1):
            for j in range(NCH):
                nc.tensor.matmul(sps0[:, g * FB:(g + 1) * FB],
                                 wk[:, j, g * KO:(g + 1) * KO], xbc(j),
                                 start=(j == 0), stop=(j == NCH - 1))
            smt_i0[g] = psq.tile([128, 512], fp32, tag="sq",
                                 name="sq_i0" + str(g))
            squash_pre(0, g, sps0[:, g * FB:(g + 1) * FB], smt_i0[g])
        squash_fin(0, 0)
        squash_fin(0, 1)

        # ================= iterations 1, 2 =================
        sps_cur = [None]

        def l_pass(g, it, filler=None):
            xv = xvp.tile([128, CI, NB * FB], fp16, tag="xv",
                          name="xv" + str(it) + str(g))
            for i in range(CI):
                vb = psv.tile([128, 3, 512], fp32, tag="vb")
                for nbg in range(3):
                    for k in range(3):
                        j = i * NB + nbg * 3 + k
                        nc.tensor.matmul(vb[:, nbg, k * FB:(k + 1) * FB],
                                         wt[:, g, j, :], a_bf[:, g, :],
                                         start=True, stop=True)
                vbf = vbp.tile([128, NB * FB], fp16, tag="vbf")
                nc.scalar.copy(
                    vbf.rearrange("p (a b) -> p a b", a=3),
                    vb[:, :, 0:3 * FB])
                # xv_i = x * V_i, contiguous fp16 at 2x
                nc.vector.tensor_tensor(
                    xv[:, i, :].rearrange("p (a f) -> p a f", a=NB),
                    xc[:, i * NB:(i + 1) * NB, :, :]
                    .rearrange("p j c b -> p j (c b)"),
                    vbf.rearrange("p (a f) -> p a f", a=NB),
                    Alu.mult)
                if filler is not None:
                    filler(i)   # independent DVE work fills the evac wait
            # tree-add over i: [8, 9*160]
            xvi = xv.rearrange("p i f -> p (i f)") \
                    .rearrange("p (i f) -> p i f", i=CI)
            t1 = smallp.tile([128, 4, NB * FB], fp16, tag="t1")
            nc.vector.tensor_tensor(t1[:], xvi[:, 0:4, :], xvi[:, 4:8, :],
                                    Alu.add)
            nc.vector.tensor_tensor(xvi[:, 0:2, :], t1[:, 0:2, :],
                                    t1[:, 2:4, :], Alu.add)
            nc.vector.tensor_tensor(xvi[:, 2, :], xvi[:, 0, :], xvi[:, 1, :],
                                    Alu.add)
            nc.scalar.activation(
                elt[:, g, :, :].rearrange("p a b -> p (a b)"), xvi[:, 2, :],
                Act.Exp, bias=(expb1 if it == 1 else expb2))

        def z_pass(g, smt):
            """Z = sum_n e^l via PE ones-matmul -> Z^2 in SBUF fp32.

            Uses the frp col-region of smt; zz is extracted before squash's
            frp matmul touches it.
            """
            zp = smt[0:1, 2 * FB:3 * FB]
            for nb in range(NB):
                nc.tensor.matmul(zp, ones16, elt[:, g, nb, :],
                                 start=(nb == 0), stop=(nb == NB - 1))
            zz = smallp.tile([1, FB], fp32, tag="zz" + str(g), name="zz")
            nc.scalar.activation(zz[:], zp, Act.Square)
            return zz

        def xe_op(g, xp, i):
            # xe_i = x * e^l, contiguous fp16 at 2x
            nc.vector.tensor_tensor(
                xp[:, i * NB:(i + 1) * NB, :],
                xc[:, i * NB:(i + 1) * NB, :, :]
                .rearrange("p j c b -> p j (c b)"),
                elt[:, g, :, :], Alu.mult)

        def s_mm(g, xp):
            for j in range(NCH):
                nc.tensor.matmul(
                    sps_cur[0][:, g * FB:(g + 1) * FB],
                    wk[:, j, g * KO:(g + 1) * KO], xp[:, j, :],
                    start=(j == 0), stop=(j == NCH - 1))

        for it in (1, 2):
            sps_cur[0] = pss.tile([KO, 2 * FB], fp32, tag="spsum",
                                  name="sps")
            # V0,V1 dense on PE; xe0 fills l1's evac-paced DVE gaps.
            # Squash PREs (all Lns) for both groups run back-to-back, then
            # both FINs (Exps) -> 2 ACT-table swaps per iteration.
            l_pass(0, it)
            xp0 = xpp.tile([128, NCH, FB], fp16, tag="xp", name="xp0")
            l_pass(1, it, filler=lambda i: xe_op(0, xp0, i))
            smt0 = psq.tile([128, 512], fp32, tag="sq", name="sq0")
            zz0 = z_pass(0, smt0)
            sq_st[0] = (zz0,)
            s_mm(0, xp0)
            squash_pre(it, 0, sps_cur[0][:, 0:FB], smt0)
            xp1 = xpp.tile([128, NCH, FB], fp16, tag="xp", name="xp1")
            for i in range(CI):
                xe_op(1, xp1, i)
            smt1 = psq.tile([128, 512], fp32, tag="sq", name="sq1")
            zz1 = z_pass(1, smt1)
            sq_st[1] = (zz1,)
            s_mm(1, xp1)
            squash_pre(it, 1, sps_cur[0][:, FB:2 * FB], smt1)
            warm(smt1, 16)
            squash_fin(it, 0)
            squash_fin(it, 1)

    nc.compile()
    return nc


def _get_compiled():
    global _compiled
    if _compiled is None:
        _compiled = _build()
    return _compiled


def _make_consts():
    c32 = np.zeros((128, C32_COLS), dtype=np.float32)
    for q in range(CG):
        c32[q * CO:(q + 1) * CO, BD0 + q] = 1.0
        c32[q, SEL0 + q * CO:SEL0 + (q + 1) * CO] = 1.0
        for g in range(2):
            c32[q, MASK50 + g * FB + q * BL:MASK50 + g * FB + (q + 1) * BL] = 1.0
    c32[0, ONE1C:ONE1C + CG] = 1.0
    c32[:, EXPB1] = 0.0
    c32[:, EXPB2] = -15.0  # keeps x*e^l in fp16 and snr in the Ln-safe range
    c32[:, NSQ] = float(N) * float(N)
    return c32


def _prep_w(route_weights: np.ndarray):
    w = np.ascontiguousarray(route_weights, dtype=np.float32)
    w5 = w.reshape(NCAPS, NB, 128, CI, CO)
    wk = np.ascontiguousarray(
        w5.transpose(2, 3, 1, 0, 4).reshape(128, NCH * NCAPS * CO)
        .astype(np.float16))
    wt = np.ascontiguousarray(
        w5.reshape(2, CG, NB, 128, CI, CO)
        .transpose(1, 5, 0, 4, 2, 3).reshape(KO, 2 * NCH * 128)
        .astype(np.float16))
    return wk, wt


def _prep_x_shard(xs: np.ndarray):
    xf = np.ascontiguousarray(
        xs.reshape(BL, NB, 128, CI).transpose(2, 3, 1, 0).reshape(128, NCH * BL))
    return xf.astype(np.float16)


def _extract_out(raw: np.ndarray) -> np.ndarray:
    """raw [KO, 2, FB] -> [NCAPS, BL, CO] diagonal blocks."""
    out = np.empty((NCAPS, BL, CO), dtype=np.float32)
    for c in range(NCAPS):
        g, cl = divmod(c, CG)
        out[c] = raw[cl * CO:(cl + 1) * CO, g, cl * BL:(cl + 1) * BL].T
    return out


def kernel(x: np.ndarray, route_weights: np.ndarray) -> np.ndarray:
    from concourse.bass_utils import run_bass_kernel_spmd

    nc = _get_compiled()
    x = np.ascontiguousarray(x, dtype=np.float32)
    wk, wt = _prep_w(route_weights)
    c32 = _make_consts()
    cb16 = np.zeros((128, CB16_COLS), dtype=np.float16)
    cb16[:, 0] = 1.0
    in_maps = []
    for ci in range(NCORES):
        xh16 = _prep_x_shard(x[ci * BL:(ci + 1) * BL])
        in_maps.append({"wkb": wk, "cb16": cb16, "xf": xh16,
                        "wt": wt, "c32": c32})
    tdir = os.environ.get("CAPS_TRACE_DIR") or None
    if tdir:
        os.makedirs(tdir, exist_ok=True)
    res = run_bass_kernel_spmd(
        nc, in_maps, list(range(NCORES)), tmpdir=tdir,
        trace=bool(int(os.environ.get("CAPS_TRACE", "0"))))
    kernel.last_res = res
    outs = [_extract_out(res.results[ci]["out"]) for ci in range(NCORES)]
    full = np.concatenate(outs, axis=1)          # [10, 256, 16]
    if res.exec_time_ns is not None:
        kernel.last_exec_time_ns = res.exec_time_ns
    return full[:, :, None, None, :].astype(np.float32)


kernel.last_exec_time_ns = None
